# revision 24
# baseline (speedup 1.0000x reference)
import os
import numpy as np
import ml_dtypes
from contextlib import ExitStack

import concourse.bass as bass
import concourse.mybir as mybir
from concourse.bass_utils import run_bass_kernel_spmd

BF16 = mybir.dt.bfloat16
F32 = mybir.dt.float32
AX = mybir.AxisListType
AF = mybir.ActivationFunctionType
OP = mybir.AluOpType

H, DH, C, T = 16, 64, 1024, 2048
NCORES = 8
NCH = T // 128  # 16 chunks of 128 tokens
EPS = 1e-5
SCALE = 8.0 / DH

bf = ml_dtypes.bfloat16

LAST_RESULTS = None
LAST_EXEC_S = None


def _bc_last(ap, n):
    return bass.AP(tensor=ap.tensor, offset=ap.offset, ap=[*ap.ap, [0, n]])


def _bc_mid(ap, n):
    return bass.AP(
        tensor=ap.tensor, offset=ap.offset, ap=[ap.ap[0], [0, n], *ap.ap[1:]]
    )


# semaphore milestone counters (phase A)
def pe_kp(ch):
    return ch * 4 + 1


def pe_vp(ch):
    return ch * 4 + 2


def pe_qp(ch, qb):
    return ch * 4 + 3 + qb


PE_A = NCH * 4  # PE count after phase A
DVE_A = NCH * 11


def dve_b(ch):
    return ch * 11


# dve idx within chunk: 1 kp-xs, 2 kp-var, 3 vp-copy,
# per qb: +1 qp-xs, +2 qp-var, +3 S, +4 x  (qb0: 4..7, qb1: 8..11)
def act_kp(ch):
    return ch * 5 + 1


def act_qp(ch, qb):
    return ch * 5 + 2 + qb * 2


def act_ex(ch, qb):
    return ch * 5 + 3 + qb * 2


def in_a(ch, j):  # j=1 kc, 2 vc, 3 qc0, 4 qc1
    return (7 + ch * 4 + j) * 16


def out_a(i):
    return (i + 1) * 16


def build_prog():
    nc = bass.Bass(use_seq_codegen=True)
    qT = nc.dram_tensor("qT", [2 * NCH * 128, 8, 128], BF16, kind="ExternalInput")
    kT = nc.dram_tensor("kT", [NCH * 128, 8, 128], BF16, kind="ExternalInput")
    vT = nc.dram_tensor("vT", [NCH * 128, 8, 128], BF16, kind="ExternalInput")
    Wall = nc.dram_tensor("Wall", [4, 8, 128, C], BF16, kind="ExternalInput")
    conb = nc.dram_tensor("conb", [128, 4, C], BF16, kind="ExternalInput")
    bo = nc.dram_tensor("bo", [128, C], F32, kind="ExternalInput")
    ident = nc.dram_tensor("ident", [128, 128], F32, kind="ExternalInput")
    out = nc.dram_tensor("out", [2 * NCH * 128, C], F32, kind="ExternalOutput")
    xbuf = nc.dram_tensor("xbuf", [2, T, C], F32, kind="Internal")

    with ExitStack() as ctx:
        _n = [0]

        def sbm(shape, dt):
            _n[0] += 1
            return ctx.enter_context(nc.sbuf_tensor(f"sb{_n[0]}", shape, dt))

        def psm(shape, dt):
            _n[0] += 1
            return ctx.enter_context(nc.psum_tensor(f"ps{_n[0]}", shape, dt))

        wq = sbm([128, 8, C], BF16)
        wk = sbm([128, 8, C], BF16)
        wv = sbm([128, 8, C], BF16)
        wo = sbm([128, 8, C], BF16)
        cons = sbm([128, 4, C], BF16)
        bos = sbm([128, C], F32)
        idt = sbm([128, 128], F32)
        epst = sbm([128, 1], F32)
        kc2 = [sbm([128, 8, 128], BF16) for _ in range(2)]
        vc2 = [sbm([128, 8, 128], BF16) for _ in range(2)]
        qc2 = [[sbm([128, 8, 128], BF16) for _ in range(2)] for _ in range(2)]
        kp_sb = sbm([128, C], BF16)
        vp_sb = sbm([128, C], BF16)
        qp_sb = sbm([128, C], BF16)
        xs = sbm([128, C], BF16)
        sq = sbm([128, C], BF16)
        t1 = sbm([128, C], BF16)
        P3 = sbm([128, C], BF16)
        Pv = sbm([128, C], BF16)
        S = sbm([128, H * H], F32)
        attn = sbm([128, H * H], BF16)
        attn2 = sbm([128, H * H], BF16)
        x = sbm([128, C], F32)
        y2 = [sbm([128, C], F32) for _ in range(2)]
        ymT2 = [sbm([128, 8, 128], BF16) for _ in range(2)]
        osb2 = [sbm([128, C], F32) for _ in range(2)]
        mu = sbm([128, H], F32)
        s2 = sbm([128, H], F32)
        m2 = sbm([128, H], F32)
        var = sbm([128, H], F32)
        lv = sbm([128, H], F32)
        rstd_k = sbm([128, H], F32)
        rstd_q = sbm([128, H], F32)
        z = sbm([128, H], F32)
        rz = sbm([128, H], F32)

        ps_kp = psm([128, C], F32)
        ps_vp = psm([128, C], F32)
        ps_qp = psm([128, C], F32)
        ps_o = psm([128, C], F32)

        sIN = ctx.enter_context(nc.semaphore("sIN"))
        sOUT = ctx.enter_context(nc.semaphore("sOUT"))
        sPE = ctx.enter_context(nc.semaphore("sPE"))
        sDVE = ctx.enter_context(nc.semaphore("sDVE"))
        sACT = ctx.enter_context(nc.semaphore("sACT"))
        sXB = ctx.enter_context(nc.semaphore("sXB"))
        sYB = ctx.enter_context(nc.semaphore("sYB"))

        gqc = cons[:, 0, :]
        bqc = cons[:, 1, :]
        gkc = cons[:, 2, :]
        bkc = cons[:, 3, :]

        blk = ctx.enter_context(nc.Block())

        def ymap_ap(qb, m, j2):
            # y tile half j2: partitions (j2*64..j2*64+64) = (jhat? d); see notes:
            # y_m[n, (j,d)] = xbuf[qb, 16n+j, 64m+d]
            # AP dims: [part n?? no] -- build: partition = n? NO:
            return None

        @blk.gpsimd
        def _(g):
            for i, wdst in enumerate((wq, wk, wv, wo)):
                g.dma_start(
                    out=wdst[:, :, :], in_=Wall[i].rearrange("b p d -> p b d")
                ).then_inc(sIN, 16)
            g.dma_start(out=cons[:, :, :], in_=conb[:, :, :]).then_inc(sIN, 16)
            g.dma_start(out=bos[:, :], in_=bo[:, :]).then_inc(sIN, 16)
            g.dma_start(out=idt[:, :], in_=ident[:, :]).then_inc(sIN, 16)
            for ch in range(NCH):
                if ch >= 2:
                    g.wait_ge(sPE, pe_qp(ch - 2, 1))
                g.dma_start(
                    out=kc2[ch % 2][:, :, :], in_=kT[ch * 128 : (ch + 1) * 128]
                ).then_inc(sIN, 16)
                g.dma_start(
                    out=vc2[ch % 2][:, :, :], in_=vT[ch * 128 : (ch + 1) * 128]
                ).then_inc(sIN, 16)
                for qb in range(2):
                    r0 = qb * (NCH * 128) + ch * 128
                    g.dma_start(
                        out=qc2[qb][ch % 2][:, :, :], in_=qT[r0 : r0 + 128]
                    ).then_inc(sIN, 16)
                for qb in range(2):
                    g.wait_ge(sDVE, dve_b(ch) + 7 + qb * 4)
                    g.dma_start(
                        out=xbuf[qb, ch * 128 : (ch + 1) * 128, :], in_=x[:, :]
                    ).then_inc(sXB, 16)
            # ---- phase B ----
            g.wait_ge(sXB, 32 * 16)
            for i in range(32):
                qb, m = divmod(i, H)
                if i >= 2:
                    g.wait_ge(sPE, PE_A + 2 * (i - 2) + 1)
                # y_m[n, (j,d)] = xbuf[qb, 16n+j, 64m+d]; partition = n
                src = bass.AP(
                    tensor=xbuf,
                    offset=qb * (T * C) + m * DH,
                    ap=[[16 * C, 128], [C, 16], [1, DH]],
                )
                g.dma_start(out=y2[i % 2][:, :], in_=src).then_inc(sYB, 16)
                if i >= 1:
                    g.wait_ge(sDVE, DVE_A + 2 * (i - 1) + 2)
                    qbp, mp = divmod(i - 1, H)
                    r0 = qbp * (NCH * 128) + mp * 128
                    g.dma_start(
                        out=out[r0 : r0 + 128], in_=osb2[(i - 1) % 2][:, :]
                    ).then_inc(sOUT, 16)
            g.wait_ge(sDVE, DVE_A + 2 * 31 + 2)
            r0 = (NCH * 128) + 15 * 128
            g.dma_start(out=out[r0 : r0 + 128], in_=osb2[31 % 2][:, :]).then_inc(
                sOUT, 16
            )

        @blk.tensor
        def _(t):
            def proj(act, w, psd):
                ins = None
                for cb in range(8):
                    for dh in range(2):
                        ins = nc.tensor.matmul(
                            psd[:, dh * 512 : (dh + 1) * 512],
                            lhsT=act[:, cb, :],
                            rhs=w[:, cb, dh * 512 : (dh + 1) * 512],
                            start=(cb == 0),
                            stop=(cb == 7),
                        )
                nc.tensor.drain()
                nc.tensor.sem_inc(sPE, 1)

            for ch in range(NCH):
                t.wait_ge(sIN, in_a(ch, 4))
                if ch >= 1:
                    t.wait_ge(sDVE, dve_b(ch - 1) + 1)
                proj(kc2[ch % 2], wk, ps_kp)
                t.wait_ge(sIN, in_a(ch, 2))
                if ch >= 1:
                    t.wait_ge(sDVE, dve_b(ch - 1) + 3)
                proj(vc2[ch % 2], wv, ps_vp)
                for qb in range(2):
                    t.wait_ge(sIN, in_a(ch, 3 + qb))
                    if qb == 1:
                        t.wait_ge(sDVE, dve_b(ch) + 4)
                    elif ch >= 1:
                        t.wait_ge(sDVE, dve_b(ch - 1) + 8)
                    proj(qc2[qb][ch % 2], wq, ps_qp)
            # ---- phase B ----
            for i in range(32):
                t.wait_ge(sYB, (i + 1) * 16)
                if i >= 1:
                    t.wait_ge(sDVE, DVE_A + 2 * (i - 1) + 1)
                ins = None
                for b in range(8):
                    ins = nc.tensor.transpose(
                        ps_kp[:, b * 128 : (b + 1) * 128],
                        y2[i % 2][:, b * 128 : (b + 1) * 128],
                        idt[:, :],
                    )
                nc.tensor.drain()
                nc.tensor.sem_inc(sPE, 1)
                t.wait_ge(sDVE, DVE_A + 2 * i + 1)
                if i >= 2:
                    t.wait_ge(sDVE, DVE_A + 2 * (i - 2) + 2)
                proj(ymT2[i % 2], wo, ps_o)

        @blk.vector
        def _(v):
            def vinc(n=1):
                v.drain()
                v.sem_inc(sDVE, n)

            v.memset(epst[:, :], EPS)
            v.wait_ge(sIN, 7 * 16)

            def ln_stats(psd):
                v.tensor_copy(out=xs[:, :], in_=psd[:, :])
                vinc()
                xs3 = xs[:, :].rearrange("p (h d) -> p h d", d=DH)
                v.reduce_sum(out=mu[:, :], in_=xs3, axis=AX.X)
                v.tensor_tensor(
                    out=sq[:, :], in0=xs[:, :], in1=xs[:, :], op=OP.mult
                )
                v.drain()
                sq3 = sq[:, :].rearrange("p (h d) -> p h d", d=DH)
                v.reduce_sum(out=s2[:, :], in_=sq3, axis=AX.X)
                v.tensor_scalar_mul(mu[:, :], mu[:, :], 1.0 / DH)
                v.drain()
                v.tensor_tensor(out=m2[:, :], in0=mu[:, :], in1=mu[:, :], op=OP.mult)
                v.tensor_scalar_mul(s2[:, :], s2[:, :], 1.0 / DH)
                v.drain()
                v.tensor_tensor(
                    out=var[:, :], in0=s2[:, :], in1=m2[:, :], op=OP.subtract
                )
                vinc()

            def ln_apply(rstd, g_sb, b_sb, dst):
                xs3 = xs[:, :].rearrange("p (h d) -> p h d", d=DH)
                t13 = t1[:, :].rearrange("p (h d) -> p h d", d=DH)
                v.tensor_tensor(
                    out=t13, in0=xs3, in1=_bc_last(mu[:, :], DH), op=OP.subtract
                )
                v.drain()
                v.tensor_tensor(
                    out=t13, in0=t13, in1=_bc_last(rstd[:, :], DH), op=OP.mult
                )
                v.drain()
                v.tensor_tensor(out=dst[:, :], in0=t1[:, :], in1=g_sb, op=OP.mult)
                v.drain()
                v.tensor_tensor(out=dst[:, :], in0=dst[:, :], in1=b_sb, op=OP.add)
                v.drain()

            for ch in range(NCH):
                v.wait_ge(sPE, pe_kp(ch))
                ln_stats(ps_kp)
                v.wait_ge(sACT, act_kp(ch))
                ln_apply(rstd_k, gkc, bkc, kp_sb)
                v.wait_ge(sPE, pe_vp(ch))
                v.tensor_copy(out=vp_sb[:, :], in_=ps_vp[:, :])
                vinc()
                kp3 = kp_sb[:, :].rearrange("p (h d) -> p h d", d=DH)
                vp3 = vp_sb[:, :].rearrange("p (h d) -> p h d", d=DH)
                for qb in range(2):
                    v.wait_ge(sPE, pe_qp(ch, qb))
                    ln_stats(ps_qp)
                    v.wait_ge(sACT, act_qp(ch, qb))
                    ln_apply(rstd_q, gqc, bqc, qp_sb)
                    qp3 = qp_sb[:, :].rearrange("p (h d) -> p h d", d=DH)
                    S3 = S[:, :].rearrange("p (h g) -> p h g", g=H)
                    P33 = P3[:, :].rearrange("p (h d) -> p h d", d=DH)
                    for gi in range(H):
                        v.tensor_tensor(
                            out=P33, in0=qp3, in1=_bc_mid(kp3[:, gi, :], H),
                            op=OP.mult,
                        )
                        v.drain()
                        v.reduce_sum(out=S3[:, :, gi], in_=P33, axis=AX.X)
                    vinc()  # S ready
                    v.wait_ge(sACT, act_ex(ch, qb))
                    at3 = attn[:, :].rearrange("p (h g) -> p h g", g=H)
                    v.reduce_sum(out=z[:, :], in_=at3, axis=AX.X)
                    v.drain()
                    v.reciprocal(rz[:, :], z[:, :])
                    v.drain()
                    at23 = attn2[:, :].rearrange("p (h g) -> p h g", g=H)
                    v.tensor_tensor(
                        out=at23, in0=at3, in1=_bc_last(rz[:, :], H), op=OP.mult
                    )
                    v.drain()
                    # wait for previous x dump to DRAM before overwriting x
                    v.wait_ge(sXB, (2 * ch + qb) * 16)
                    x3 = x[:, :].rearrange("p (h d) -> p h d", d=DH)
                    Pv3 = Pv[:, :].rearrange("p (h d) -> p h d", d=DH)
                    for gi in range(H):
                        if gi == 0:
                            v.tensor_tensor(
                                out=x3, in0=_bc_mid(vp3[:, 0, :], H),
                                in1=_bc_last(at23[:, :, 0], DH), op=OP.mult,
                            )
                            v.drain()
                        else:
                            v.tensor_tensor(
                                out=Pv3, in0=_bc_mid(vp3[:, gi, :], H),
                                in1=_bc_last(at23[:, :, gi], DH), op=OP.mult,
                            )
                            v.drain()
                            v.tensor_tensor(out=x3, in0=x3, in1=Pv3, op=OP.add)
                            v.drain()
                    vinc()  # x ready
            # ---- phase B ----
            for i in range(32):
                v.wait_ge(sPE, PE_A + 2 * i + 1)
                v.tensor_copy(out=ymT2[i % 2][:, :, :], in_=ps_kp[:, :])
                vinc()
                v.wait_ge(sPE, PE_A + 2 * i + 2)
                if i >= 2:
                    v.wait_ge(sOUT, out_a(i - 2))
                v.tensor_tensor(
                    out=osb2[i % 2][:, :], in0=ps_o[:, :], in1=bos[:, :], op=OP.add
                )
                vinc()

        @blk.scalar
        def _(a):
            for ch in range(NCH):
                a.wait_ge(sDVE, dve_b(ch) + 2)
                a.activation(
                    out=lv[:, :], in_=var[:, :], func=AF.Ln, bias=epst[:, 0:1]
                )
                a.drain()
                a.activation(
                    out=rstd_k[:, :], in_=lv[:, :], func=AF.Exp, scale=-0.5
                )
                a.drain()
                a.sem_inc(sACT, 1)
                for qb in range(2):
                    a.wait_ge(sDVE, dve_b(ch) + 5 + qb * 4)
                    a.activation(
                        out=lv[:, :], in_=var[:, :], func=AF.Ln, bias=epst[:, 0:1]
                    )
                    a.drain()
                    a.activation(
                        out=rstd_q[:, :], in_=lv[:, :], func=AF.Exp, scale=-0.5
                    )
                    a.drain()
                    a.sem_inc(sACT, 1)
                    a.wait_ge(sDVE, dve_b(ch) + 6 + qb * 4)
                    a.activation(out=attn[:, :], in_=S[:, :], func=AF.Exp)
                    a.drain()
                    a.sem_inc(sACT, 1)

    return nc


_PROGS = {}


def _get_prog():
    if "p" not in _PROGS:
        _PROGS["p"] = build_prog()
    return _PROGS["p"]


def _prep_T(a):
    r = a.reshape(NCH, 128, 8, 128)  # [t1, t2, cb, p]
    return np.ascontiguousarray(r.transpose(0, 3, 2, 1).astype(bf)).reshape(
        NCH * 128, 8, 128
    )


def _kernel_device(q, k, v, Wq, Wk, Wv, Wo, bo_, gamma, beta):
    nc = _get_prog()

    Wall = np.stack(
        [np.ascontiguousarray(W.T).reshape(8, 128, C) for W in (Wq, Wk, Wv, Wo)]
    ).astype(bf)
    gq = np.tile(gamma * SCALE, H)
    bq = np.tile(beta * SCALE, H)
    gk = np.tile(gamma, H)
    bk = np.tile(beta, H)
    conb = np.ascontiguousarray(
        np.broadcast_to(np.stack([gq, bq, gk, bk], axis=0), (128, 4, C))
    ).astype(bf)
    bof = np.ascontiguousarray(np.broadcast_to(bo_, (128, C)), np.float32)
    ident = np.eye(128, dtype=np.float32)

    in_maps = []
    for c in range(NCORES):
        qTc = np.concatenate([_prep_T(q[c]), _prep_T(q[c + 8])])
        kTc = _prep_T(k[c % 4])
        vTc = _prep_T(v[c % 4])
        in_maps.append(
            dict(qT=qTc, kT=kTc, vT=vTc, Wall=Wall, conb=conb, bo=bof, ident=ident)
        )

    global LAST_RESULTS, LAST_EXEC_S
    import time

    t0 = time.time()
    res = run_bass_kernel_spmd(nc, in_maps, core_ids=list(range(NCORES)))
    LAST_EXEC_S = time.time() - t0
    LAST_RESULTS = res
    out = np.empty((16, T, C), np.float32)
    for c in range(NCORES):
        oc = np.asarray(res.results[c]["out"]).reshape(2, T, C)
        out[c] = oc[0]
        out[c + 8] = oc[1]
    return out


def kernel(q, k, v, Wq, Wk, Wv, Wo, bo, gamma, beta):
    args = [
        np.asarray(a, np.float32)
        for a in (q, k, v, Wq, Wk, Wv, Wo, bo, gamma, beta)
    ]
    return _kernel_device(*args)


# revision 26
# speedup vs baseline: 2.0631x; 2.0631x over previous
import os
import numpy as np
import ml_dtypes
from contextlib import ExitStack

import jax

try:
    jax.config.update("jax_compilation_cache_dir", "/tmp/bass_jax_cache")
    jax.config.update("jax_persistent_cache_min_compile_time_secs", 1.0)
    jax.config.update("jax_persistent_cache_min_entry_size_bytes", 0)
except Exception:
    pass

import concourse.bass as bass
import concourse.mybir as mybir
from concourse.bass_utils import run_bass_kernel_spmd

BF16 = mybir.dt.bfloat16
F32 = mybir.dt.float32
AX = mybir.AxisListType
AF = mybir.ActivationFunctionType
OP = mybir.AluOpType

H, DH, C, T = 16, 64, 1024, 2048
NCORES = 8
NCH = T // 128  # 16 chunks of 128 tokens
EPS = 1e-5
SCALE = 8.0 / DH

bf = ml_dtypes.bfloat16

LAST_RESULTS = None
LAST_EXEC_S = None


def _bc_last(ap, n):
    return bass.AP(tensor=ap.tensor, offset=ap.offset, ap=[*ap.ap, [0, n]])


def _bc_mid(ap, n):
    return bass.AP(
        tensor=ap.tensor, offset=ap.offset, ap=[ap.ap[0], [0, n], *ap.ap[1:]]
    )


# semaphore milestone counters (phase A)
def pe_kp(ch):
    return ch * 4 + 1


def pe_vp(ch):
    return ch * 4 + 2


def pe_qp(ch, qb):
    return ch * 4 + 3 + qb


PE_A = NCH * 4  # PE count after phase A
DVE_A = NCH * 11


def dve_b(ch):
    return ch * 11


# dve idx within chunk: 1 kp-xs, 2 kp-var, 3 vp-copy,
# per qb: +1 qp-xs, +2 qp-var, +3 S, +4 x  (qb0: 4..7, qb1: 8..11)
def act_kp(ch):
    return ch * 5 + 1


def act_qp(ch, qb):
    return ch * 5 + 2 + qb * 2


def act_ex(ch, qb):
    return ch * 5 + 3 + qb * 2


def in_a(ch, j):  # j=1 kc, 2 vc, 3 qc0, 4 qc1
    return (7 + ch * 4 + j) * 16


def out_a(i):
    return (i + 1) * 16


def build_prog():
    nc = bass.Bass(use_seq_codegen=True)
    qT = nc.dram_tensor("qT", [2 * NCH * 128, 8, 128], BF16, kind="ExternalInput")
    kT = nc.dram_tensor("kT", [NCH * 128, 8, 128], BF16, kind="ExternalInput")
    vT = nc.dram_tensor("vT", [NCH * 128, 8, 128], BF16, kind="ExternalInput")
    Wall = nc.dram_tensor("Wall", [4, 8, 128, C], BF16, kind="ExternalInput")
    conb = nc.dram_tensor("conb", [128, 4, C], BF16, kind="ExternalInput")
    bo = nc.dram_tensor("bo", [128, C], F32, kind="ExternalInput")
    ident = nc.dram_tensor("ident", [128, 128], F32, kind="ExternalInput")
    out = nc.dram_tensor("out", [2 * NCH * 128, C], BF16, kind="ExternalOutput")
    xbuf = nc.dram_tensor("xbuf", [2, T, C], F32, kind="Internal")

    with ExitStack() as ctx:
        _n = [0]

        def sbm(shape, dt):
            _n[0] += 1
            return ctx.enter_context(nc.sbuf_tensor(f"sb{_n[0]}", shape, dt))

        def psm(shape, dt):
            _n[0] += 1
            return ctx.enter_context(nc.psum_tensor(f"ps{_n[0]}", shape, dt))

        wq = sbm([128, 8, C], BF16)
        wk = sbm([128, 8, C], BF16)
        wv = sbm([128, 8, C], BF16)
        wo = sbm([128, 8, C], BF16)
        cons = sbm([128, 4, C], BF16)
        bos = sbm([128, C], F32)
        idt = sbm([128, 128], F32)
        epst = sbm([128, 1], F32)
        kc2 = [sbm([128, 8, 128], BF16) for _ in range(2)]
        vc2 = [sbm([128, 8, 128], BF16) for _ in range(2)]
        qc2 = [[sbm([128, 8, 128], BF16) for _ in range(2)] for _ in range(2)]
        kp_sb = sbm([128, C], BF16)
        vp_sb = sbm([128, C], BF16)
        qp_sb = sbm([128, C], BF16)
        xs = sbm([128, C], BF16)
        sq = sbm([128, C], BF16)
        t1 = sbm([128, C], BF16)
        P3 = sbm([128, C], BF16)
        Pv = sbm([128, C], BF16)
        S = sbm([128, H * H], F32)
        attn = sbm([128, H * H], BF16)
        attn2 = sbm([128, H * H], BF16)
        x = sbm([128, C], F32)
        y2 = [sbm([128, C], F32) for _ in range(2)]
        ymT2 = [sbm([128, 8, 128], BF16) for _ in range(2)]
        osb2 = [sbm([128, C], BF16) for _ in range(2)]
        mu = sbm([128, H], F32)
        s2 = sbm([128, H], F32)
        m2 = sbm([128, H], F32)
        var = sbm([128, H], F32)
        lv = sbm([128, H], F32)
        rstd_k = sbm([128, H], F32)
        rstd_q = sbm([128, H], F32)
        z = sbm([128, H], F32)
        rz = sbm([128, H], F32)

        ps_kp = psm([128, C], F32)
        ps_vp = psm([128, C], F32)
        ps_qp = psm([128, C], F32)
        ps_o = psm([128, C], F32)

        sIN = ctx.enter_context(nc.semaphore("sIN"))
        sOUT = ctx.enter_context(nc.semaphore("sOUT"))
        sPE = ctx.enter_context(nc.semaphore("sPE"))
        sDVE = ctx.enter_context(nc.semaphore("sDVE"))
        sACT = ctx.enter_context(nc.semaphore("sACT"))
        sXB = ctx.enter_context(nc.semaphore("sXB"))
        sYB = ctx.enter_context(nc.semaphore("sYB"))

        gqc = cons[:, 0, :]
        bqc = cons[:, 1, :]
        gkc = cons[:, 2, :]
        bkc = cons[:, 3, :]

        blk = ctx.enter_context(nc.Block())

        def ymap_ap(qb, m, j2):
            # y tile half j2: partitions (j2*64..j2*64+64) = (jhat? d); see notes:
            # y_m[n, (j,d)] = xbuf[qb, 16n+j, 64m+d]
            # AP dims: [part n?? no] -- build: partition = n? NO:
            return None

        @blk.gpsimd
        def _(g):
            for i, wdst in enumerate((wq, wk, wv, wo)):
                g.dma_start(
                    out=wdst[:, :, :], in_=Wall[i].rearrange("b p d -> p b d")
                ).then_inc(sIN, 16)
            g.dma_start(out=cons[:, :, :], in_=conb[:, :, :]).then_inc(sIN, 16)
            g.dma_start(out=bos[:, :], in_=bo[:, :]).then_inc(sIN, 16)
            g.dma_start(out=idt[:, :], in_=ident[:, :]).then_inc(sIN, 16)
            for ch in range(NCH):
                if ch >= 2:
                    g.wait_ge(sPE, pe_qp(ch - 2, 1))
                g.dma_start(
                    out=kc2[ch % 2][:, :, :], in_=kT[ch * 128 : (ch + 1) * 128]
                ).then_inc(sIN, 16)
                g.dma_start(
                    out=vc2[ch % 2][:, :, :], in_=vT[ch * 128 : (ch + 1) * 128]
                ).then_inc(sIN, 16)
                for qb in range(2):
                    r0 = qb * (NCH * 128) + ch * 128
                    g.dma_start(
                        out=qc2[qb][ch % 2][:, :, :], in_=qT[r0 : r0 + 128]
                    ).then_inc(sIN, 16)
                for qb in range(2):
                    g.wait_ge(sDVE, dve_b(ch) + 7 + qb * 4)
                    g.dma_start(
                        out=xbuf[qb, ch * 128 : (ch + 1) * 128, :], in_=x[:, :]
                    ).then_inc(sXB, 16)
            # ---- phase B ----
            g.wait_ge(sXB, 32 * 16)
            for i in range(32):
                qb, m = divmod(i, H)
                if i >= 2:
                    g.wait_ge(sPE, PE_A + 2 * (i - 2) + 1)
                # y_m[n, (j,d)] = xbuf[qb, 16n+j, 64m+d]; partition = n
                src = bass.AP(
                    tensor=xbuf,
                    offset=qb * (T * C) + m * DH,
                    ap=[[16 * C, 128], [C, 16], [1, DH]],
                )
                g.dma_start(out=y2[i % 2][:, :], in_=src).then_inc(sYB, 16)
                if i >= 1:
                    g.wait_ge(sDVE, DVE_A + 2 * (i - 1) + 2)
                    qbp, mp = divmod(i - 1, H)
                    r0 = qbp * (NCH * 128) + mp * 128
                    g.dma_start(
                        out=out[r0 : r0 + 128], in_=osb2[(i - 1) % 2][:, :]
                    ).then_inc(sOUT, 16)
            g.wait_ge(sDVE, DVE_A + 2 * 31 + 2)
            r0 = (NCH * 128) + 15 * 128
            g.dma_start(out=out[r0 : r0 + 128], in_=osb2[31 % 2][:, :]).then_inc(
                sOUT, 16
            )

        @blk.tensor
        def _(t):
            def proj(act, w, psd):
                ins = None
                for cb in range(8):
                    for dh in range(2):
                        ins = nc.tensor.matmul(
                            psd[:, dh * 512 : (dh + 1) * 512],
                            lhsT=act[:, cb, :],
                            rhs=w[:, cb, dh * 512 : (dh + 1) * 512],
                            start=(cb == 0),
                            stop=(cb == 7),
                        )
                nc.tensor.drain()
                nc.tensor.sem_inc(sPE, 1)

            for ch in range(NCH):
                t.wait_ge(sIN, in_a(ch, 4))
                if ch >= 1:
                    t.wait_ge(sDVE, dve_b(ch - 1) + 1)
                proj(kc2[ch % 2], wk, ps_kp)
                t.wait_ge(sIN, in_a(ch, 2))
                if ch >= 1:
                    t.wait_ge(sDVE, dve_b(ch - 1) + 3)
                proj(vc2[ch % 2], wv, ps_vp)
                for qb in range(2):
                    t.wait_ge(sIN, in_a(ch, 3 + qb))
                    if qb == 1:
                        t.wait_ge(sDVE, dve_b(ch) + 4)
                    elif ch >= 1:
                        t.wait_ge(sDVE, dve_b(ch - 1) + 8)
                    proj(qc2[qb][ch % 2], wq, ps_qp)
            # ---- phase B ----
            for i in range(32):
                t.wait_ge(sYB, (i + 1) * 16)
                if i >= 1:
                    t.wait_ge(sDVE, DVE_A + 2 * (i - 1) + 1)
                ins = None
                for b in range(8):
                    ins = nc.tensor.transpose(
                        ps_kp[:, b * 128 : (b + 1) * 128],
                        y2[i % 2][:, b * 128 : (b + 1) * 128],
                        idt[:, :],
                    )
                nc.tensor.drain()
                nc.tensor.sem_inc(sPE, 1)
                t.wait_ge(sDVE, DVE_A + 2 * i + 1)
                if i >= 2:
                    t.wait_ge(sDVE, DVE_A + 2 * (i - 2) + 2)
                proj(ymT2[i % 2], wo, ps_o)

        @blk.vector
        def _(v):
            def vinc(n=1):
                v.drain()
                v.sem_inc(sDVE, n)

            v.memset(epst[:, :], EPS)
            v.wait_ge(sIN, 7 * 16)

            def ln_stats(psd):
                v.tensor_copy(out=xs[:, :], in_=psd[:, :])
                vinc()
                xs3 = xs[:, :].rearrange("p (h d) -> p h d", d=DH)
                v.reduce_sum(out=mu[:, :], in_=xs3, axis=AX.X)
                v.tensor_tensor(
                    out=sq[:, :], in0=xs[:, :], in1=xs[:, :], op=OP.mult
                )
                sq3 = sq[:, :].rearrange("p (h d) -> p h d", d=DH)
                v.reduce_sum(out=s2[:, :], in_=sq3, axis=AX.X)
                v.tensor_scalar_mul(mu[:, :], mu[:, :], 1.0 / DH)
                v.drain()
                v.tensor_tensor(out=m2[:, :], in0=mu[:, :], in1=mu[:, :], op=OP.mult)
                v.tensor_scalar_mul(s2[:, :], s2[:, :], 1.0 / DH)
                v.drain()
                v.tensor_tensor(
                    out=var[:, :], in0=s2[:, :], in1=m2[:, :], op=OP.subtract
                )
                vinc()

            def ln_apply(rstd, g_sb, b_sb, dst):
                xs3 = xs[:, :].rearrange("p (h d) -> p h d", d=DH)
                t13 = t1[:, :].rearrange("p (h d) -> p h d", d=DH)
                v.tensor_tensor(
                    out=t13, in0=xs3, in1=_bc_last(mu[:, :], DH), op=OP.subtract
                )
                v.tensor_tensor(
                    out=t13, in0=t13, in1=_bc_last(rstd[:, :], DH), op=OP.mult
                )
                v.tensor_tensor(out=dst[:, :], in0=t1[:, :], in1=g_sb, op=OP.mult)
                v.tensor_tensor(out=dst[:, :], in0=dst[:, :], in1=b_sb, op=OP.add)

            for ch in range(NCH):
                v.wait_ge(sPE, pe_kp(ch))
                ln_stats(ps_kp)
                v.wait_ge(sACT, act_kp(ch))
                ln_apply(rstd_k, gkc, bkc, kp_sb)
                v.wait_ge(sPE, pe_vp(ch))
                v.tensor_copy(out=vp_sb[:, :], in_=ps_vp[:, :])
                vinc()
                kp3 = kp_sb[:, :].rearrange("p (h d) -> p h d", d=DH)
                vp3 = vp_sb[:, :].rearrange("p (h d) -> p h d", d=DH)
                for qb in range(2):
                    v.wait_ge(sPE, pe_qp(ch, qb))
                    ln_stats(ps_qp)
                    v.wait_ge(sACT, act_qp(ch, qb))
                    ln_apply(rstd_q, gqc, bqc, qp_sb)
                    qp3 = qp_sb[:, :].rearrange("p (h d) -> p h d", d=DH)
                    S3 = S[:, :].rearrange("p (h g) -> p h g", g=H)
                    P33 = P3[:, :].rearrange("p (h d) -> p h d", d=DH)
                    for gi in range(H):
                        v.tensor_tensor(
                            out=P33, in0=qp3, in1=_bc_mid(kp3[:, gi, :], H),
                            op=OP.mult,
                        )
                        v.reduce_sum(out=S3[:, :, gi], in_=P33, axis=AX.X)
                    vinc()  # S ready
                    v.wait_ge(sACT, act_ex(ch, qb))
                    at3 = attn[:, :].rearrange("p (h g) -> p h g", g=H)
                    v.reduce_sum(out=z[:, :], in_=at3, axis=AX.X)
                    v.drain()
                    v.reciprocal(rz[:, :], z[:, :])
                    v.drain()
                    at23 = attn2[:, :].rearrange("p (h g) -> p h g", g=H)
                    v.tensor_tensor(
                        out=at23, in0=at3, in1=_bc_last(rz[:, :], H), op=OP.mult
                    )
                    v.drain()
                    # wait for previous x dump to DRAM before overwriting x
                    v.wait_ge(sXB, (2 * ch + qb) * 16)
                    x3 = x[:, :].rearrange("p (h d) -> p h d", d=DH)
                    Pv3 = Pv[:, :].rearrange("p (h d) -> p h d", d=DH)
                    for gi in range(H):
                        if gi == 0:
                            v.tensor_tensor(
                                out=x3, in0=_bc_mid(vp3[:, 0, :], H),
                                in1=_bc_last(at23[:, :, 0], DH), op=OP.mult,
                            )
                        else:
                            v.tensor_tensor(
                                out=Pv3, in0=_bc_mid(vp3[:, gi, :], H),
                                in1=_bc_last(at23[:, :, gi], DH), op=OP.mult,
                            )
                            v.tensor_tensor(out=x3, in0=x3, in1=Pv3, op=OP.add)
                    vinc()  # x ready
            # ---- phase B ----
            for i in range(32):
                v.wait_ge(sPE, PE_A + 2 * i + 1)
                v.tensor_copy(out=ymT2[i % 2][:, :, :], in_=ps_kp[:, :])
                vinc()
                v.wait_ge(sPE, PE_A + 2 * i + 2)
                if i >= 2:
                    v.wait_ge(sOUT, out_a(i - 2))
                v.tensor_tensor(
                    out=osb2[i % 2][:, :], in0=ps_o[:, :], in1=bos[:, :], op=OP.add
                )
                vinc()

        @blk.scalar
        def _(a):
            for ch in range(NCH):
                a.wait_ge(sDVE, dve_b(ch) + 2)
                a.activation(
                    out=lv[:, :], in_=var[:, :], func=AF.Ln, bias=epst[:, 0:1]
                )
                a.drain()
                a.activation(
                    out=rstd_k[:, :], in_=lv[:, :], func=AF.Exp, scale=-0.5
                )
                a.drain()
                a.sem_inc(sACT, 1)
                for qb in range(2):
                    a.wait_ge(sDVE, dve_b(ch) + 5 + qb * 4)
                    a.activation(
                        out=lv[:, :], in_=var[:, :], func=AF.Ln, bias=epst[:, 0:1]
                    )
                    a.drain()
                    a.activation(
                        out=rstd_q[:, :], in_=lv[:, :], func=AF.Exp, scale=-0.5
                    )
                    a.drain()
                    a.sem_inc(sACT, 1)
                    a.wait_ge(sDVE, dve_b(ch) + 6 + qb * 4)
                    a.activation(out=attn[:, :], in_=S[:, :], func=AF.Exp)
                    a.drain()
                    a.sem_inc(sACT, 1)

    return nc


_PROGS = {}


def _get_prog():
    if "p" not in _PROGS:
        _PROGS["p"] = build_prog()
    return _PROGS["p"]


def _prep_T(a):
    r = a.reshape(NCH, 128, 8, 128)  # [t1, t2, cb, p]
    return np.ascontiguousarray(r.transpose(0, 3, 2, 1).astype(bf)).reshape(
        NCH * 128, 8, 128
    )


def _kernel_device(q, k, v, Wq, Wk, Wv, Wo, bo_, gamma, beta):
    nc = _get_prog()

    Wall = np.stack(
        [np.ascontiguousarray(W.T).reshape(8, 128, C) for W in (Wq, Wk, Wv, Wo)]
    ).astype(bf)
    gq = np.tile(gamma * SCALE, H)
    bq = np.tile(beta * SCALE, H)
    gk = np.tile(gamma, H)
    bk = np.tile(beta, H)
    conb = np.ascontiguousarray(
        np.broadcast_to(np.stack([gq, bq, gk, bk], axis=0), (128, 4, C))
    ).astype(bf)
    bof = np.ascontiguousarray(np.broadcast_to(bo_, (128, C)), np.float32)
    ident = np.eye(128, dtype=np.float32)

    in_maps = []
    for c in range(NCORES):
        qTc = np.concatenate([_prep_T(q[c]), _prep_T(q[c + 8])])
        kTc = _prep_T(k[c % 4])
        vTc = _prep_T(v[c % 4])
        in_maps.append(
            dict(qT=qTc, kT=kTc, vT=vTc, Wall=Wall, conb=conb, bo=bof, ident=ident)
        )

    global LAST_RESULTS, LAST_EXEC_S
    import time

    t0 = time.time()
    res = run_bass_kernel_spmd(nc, in_maps, core_ids=list(range(NCORES)))
    LAST_EXEC_S = time.time() - t0
    LAST_RESULTS = res
    out = np.empty((16, T, C), np.float32)
    for c in range(NCORES):
        oc = np.asarray(res.results[c]["out"]).astype(np.float32).reshape(2, T, C)
        out[c] = oc[0]
        out[c + 8] = oc[1]
    return out


def kernel(q, k, v, Wq, Wk, Wv, Wo, bo, gamma, beta):
    args = [
        np.asarray(a, np.float32)
        for a in (q, k, v, Wq, Wk, Wv, Wo, bo, gamma, beta)
    ]
    return _kernel_device(*args)
